# revision 14
# baseline (speedup 1.0000x reference)
"""Bahdanau attention kernel for Trainium2 (8 NeuronCores, data-parallel over batch).

Reference computation (per batch row b):
    pq      = query @ Wq.T                       # (B, AD)
    hidden  = tanh(pq[:, None, :] + processed_memory)   # (B, T, AD)
    e       = einsum('btd,d->bt', hidden, v)     # (B, T)
    e       = where(mask, -1e30, e)
    out     = softmax(e, axis=1)

Key observation: ~50% of positions are masked (mask True -> softmax weight
exactly 0), so the host gathers only the unmasked columns of
processed_memory per batch (max count 2126 of 4096 for the reference data)
into a compact [AD, Tc=2176] slab, and scatters the compact softmax back
into the full [B, T] output (zeros at masked positions).  This halves HBM
traffic, tanh work and matmul work on the device.

Device strategy (per core, 8 batches):
  * gathered pm is host-transposed to [b, 2, 128, Tc] fp16 (AD=256 split in
    two partition blocks).  The per-d "+pq" add folds into the ScalarE tanh
    as a per-partition activation bias, v-weighted reduction over d runs on
    TensorE.
  * Energies accumulate directly into a PSUM tile [8, 2048] (+[8, 128]
    tail) with batch = partition: each matmul uses a one-hot stationary
    [128, 8] whose column b holds v (other columns zero), so it adds
    v.h on row b and +0 on the other rows.  No PSUM->SBUF strip copies, no
    SBUF relayout DMAs.
  * Pad columns (count_b..Tc) hold -10*sign(v_d): tanh saturates to
    -sign(v_d), so each pad contributes exp(-sum|v|) ~ 3e-6 to the softmax
    sum (rel err ~1e-7); pad outputs are discarded by the host scatter.
  * Softmax tail with batch on partitions: exp reads PSUM directly,
    row-sum + reciprocal + rescale on DVE, direct DMA to out[b, t].
"""

import sys

if "/opt/trn_rl_repo" not in sys.path:
    sys.path.insert(0, "/opt/trn_rl_repo")

import numpy as np

import concourse.bacc as bacc
import concourse.bass as bass
import concourse.tile as tile
from concourse import mybir
from concourse.bass_utils import run_bass_kernel_spmd

B, T, QD, AD = 64, 4096, 1024, 256
NCORES = 8
BLOC = B // NCORES  # batches per core
KB = QD // 128      # k-blocks for the pq matmul
DB = AD // 128      # d-blocks (partition blocks of AD)
F32 = mybir.dt.float32
F16 = mybir.dt.float16

MAIN = 2048         # main energies region (4 PSUM banks of 512 fp32)
TAIL = 80           # tail region (counts range ~1984..2126 <= 2128)
TC = MAIN + TAIL    # compact (gathered) time extent per batch
NCH = MAIN // 512   # 512-col chunks in the main region


def build_nc() -> bass.Bass:
    # Bacc (not plain Bass): its nop/event-semaphore lowering passes are what
    # let Tile-scheduled instructions carry multiple semaphore waits.
    nc = bacc.Bacc(None, target_bir_lowering=False)

    pm_g = nc.declare_dram_parameter("pm_g", [BLOC, DB, 128, TC], F16, isOutput=False)
    # qT16[p, kb*BLOC + b] = query[b, kb*128 + p]  (host-packed, partition-major)
    qT16 = nc.declare_dram_parameter("qT16", [128, KB * BLOC], F16, isOutput=False)
    WqT16 = nc.declare_dram_parameter("WqT16", [QD, AD], F16, isOutput=False)
    # one-hot stationaries: oh[p, db, b, m] = v[db*128+p] if m == b else 0
    ohd = nc.declare_dram_parameter("oh", [128, DB, BLOC, BLOC], F16, isOutput=False)
    out = nc.declare_dram_parameter("out", [BLOC, TC], F16, isOutput=True)
    rsum = nc.declare_dram_parameter("rsum", [BLOC, 1], F32, isOutput=True)

    Tanh = mybir.ActivationFunctionType.Tanh
    Exp = mybir.ActivationFunctionType.Exp

    # energy matmul chunks: 512-col chunks (one PSUM bank each) + 128 tail
    chunks = [(c * 512, (c + 1) * 512) for c in range(NCH)] + [(MAIN, TC)]

    with tile.TileContext(nc) as tc:
        with (
            tc.tile_pool(name="singles", bufs=1) as singles,
            tc.tile_pool(name="pm", bufs=8) as pm_pool,
            tc.tile_pool(name="hid", bufs=6) as hid_pool,
            tc.tile_pool(name="epsum", bufs=1, space="PSUM") as epsum_pool,
            tc.tile_pool(name="ppsum", bufs=2, space="PSUM") as ppsum_pool,
        ):
            # ---- constant loads, ordered to minimize time-to-first-tanh:
            # wq d-block 0 + qt gate the d0 pq matmuls; the first pm tile is
            # split in half so tanh b0/d0 starts as early as possible ----
            wq_sb = [singles.tile([128, KB, 128], F16, name=f"wq{d}") for d in range(DB)]
            qt_sb = singles.tile([128, KB, BLOC], F16)
            pm_sbs = {}
            HSPL = 1024
            pm_sbs[(0, 0)] = pm_pool.tile([128, TC], F16, name="pm_sb", tag="pm_sb")
            nc.sync.dma_start(out=pm_sbs[(0, 0)][:, 0:HSPL], in_=pm_g[0, 0][:, 0:HSPL])
            nc.sync.dma_start(
                out=wq_sb[0],
                in_=WqT16[:, 0:128].rearrange("(kb p) d -> p kb d", p=128),
            )
            nc.sync.dma_start(
                out=qt_sb, in_=qT16[:, :].rearrange("p (kb b) -> p kb b", b=BLOC)
            )
            nc.sync.dma_start(
                out=wq_sb[1],
                in_=WqT16[:, 128:256].rearrange("(kb p) d -> p kb d", p=128),
            )
            nc.sync.dma_start(out=pm_sbs[(0, 0)][:, HSPL:TC], in_=pm_g[0, 0][:, HSPL:TC])
            pm_sbs[(0, 1)] = pm_pool.tile([128, TC], F16, name="pm_sb", tag="pm_sb")
            nc.sync.dma_start(out=pm_sbs[(0, 1)], in_=pm_g[0, 1])
            oh_sb = singles.tile([128, DB, BLOC, BLOC], F16)
            nc.sync.dma_start(out=oh_sb, in_=ohd[:, :, :, :])
            pm_m = {}
            for b in range(1, BLOC):
                if b < BLOC - 1:
                    # merged per-batch tile: DVE adds the pq bias in place,
                    # then one bias-free tanh covers both d-blocks
                    pm_m[b] = pm_pool.tile([128, DB, TC], F16, name="pm_m", tag="pm_m")
                    nc.sync.dma_start(
                        out=pm_m[b], in_=pm_g[b].rearrange("db p t -> p db t")
                    )
                else:
                    for d in range(DB):
                        pm_sbs[(b, d)] = pm_pool.tile([128, TC], F16, name="pm_sb", tag="pm_sb")
                        nc.sync.dma_start(out=pm_sbs[(b, d)], in_=pm_g[b, d])

            # ---- energies PSUM accumulator (batch = partition row) ----
            # [8, 2128] fp32 spans 4.2 banks; every matmul chunk below stays
            # inside a single 512-fp32 bank region.
            ep = epsum_pool.tile([BLOC, TC], F32, tag="ep")
            pq_sb = singles.tile([128, DB, BLOC], F32)

            def emit_pq(d):
                # pq = Wq @ query.T for one d-block, laid out [d % 128, b]
                ppq = ppsum_pool.tile([128, BLOC], F32, tag="ppq", name="ppq")
                for k in range(KB):
                    nc.tensor.matmul(
                        ppq,
                        lhsT=wq_sb[d][:, k, :],
                        rhs=qt_sb[:, k, :],
                        start=(k == 0),
                        stop=(k == KB - 1),
                    )
                nc.scalar.copy(pq_sb[:, d, :], ppq)

            def emit_bd(b, d, split):
                # biased tanh for one (batch, d-block) + its energy matmuls
                h = hid_pool.tile([128, TC], F16, name="h", tag="h")
                first = b == 0 and d == 0
                last = b == BLOC - 1 and d == DB - 1
                parts = ((0, HSPL), (HSPL, TC)) if split else ((0, TC),)
                for lo2, hi2 in parts:
                    nc.scalar.activation(
                        out=h[:, lo2:hi2],
                        in_=pm_sbs[(b, d)][:, lo2:hi2],
                        func=Tanh,
                        bias=pq_sb[:, d, b : b + 1],
                        scale=1.0,
                    )
                for lo, hi in chunks:
                    nc.tensor.matmul(
                        ep[:, lo:hi],
                        lhsT=oh_sb[:, d, b, :],
                        rhs=h[:, lo:hi],
                        start=first,
                        stop=last,
                    )

            # batch 0: pq d1 is computed between the two d-blocks so the
            # d1 bias copy never delays the first tanh (ACT is in-order)
            emit_pq(0)
            emit_bd(0, 0, split=True)
            emit_pq(1)
            emit_bd(0, 1, split=False)

            # batches 1..6: DVE adds the pq bias in place, then one merged
            # bias-free tanh covers both d-blocks (saves ACT instr overhead)
            for b in range(1, BLOC - 1):
                for d in range(DB):
                    nc.vector.tensor_scalar_add(
                        out=pm_m[b][:, d, :],
                        in0=pm_m[b][:, d, :],
                        scalar1=pq_sb[:, d, b : b + 1],
                    )
                hm = hid_pool.tile([128, DB, TC], F16, name="hm", tag="hm")
                nc.scalar.activation(out=hm, in_=pm_m[b], func=Tanh)
                for d in range(DB):
                    for lo, hi in chunks:
                        nc.tensor.matmul(
                            ep[:, lo:hi],
                            lhsT=oh_sb[:, d, b, :],
                            rhs=hm[:, d, lo:hi],
                            start=False,
                            stop=False,
                        )

            # batch 7: split the last tanh so its chunk-0/1 matmuls overlap
            # the second tanh half (shortens the matmul tail before exp)
            emit_bd(BLOC - 1, 0, split=False)
            emit_bd(BLOC - 1, 1, split=True)

            # ---- softmax tail: batch already on partitions; exp reads PSUM
            # directly and its row-sum is fused via accum_out ----
            # exp(e) <= exp(~3) ~ 20 for this data (|e| worst-case bound 12.8
            # needs 17 sigma) so fp16 work is safe and gets DVE 2x rescale
            work = singles.tile([BLOC, TC], F16)
            outw = singles.tile([BLOC, TC], F16)
            cs = singles.tile([BLOC, 1], F32)
            nc.scalar.activation(out=work, in_=ep, func=Exp, accum_out=cs)
            nc.sync.dma_start(out=rsum[:, :], in_=cs)
            rinv = singles.tile([BLOC, 1], F32)
            nc.vector.reciprocal(out=rinv, in_=cs)
            nc.vector.tensor_scalar_mul(out=outw, in0=work, scalar1=rinv)
            nc.sync.dma_start(out=out[:, :], in_=outw)

    # Run the Bacc lowering passes (move_matmul_waits_to_ldweights,
    # generate_event_semaphores, alloc_regs, ...) — run_bass_via_pjrt takes
    # the module as-is and walrus rejects unlowered multi-wait instructions.
    nc.finalize()
    return nc


_CACHE: dict = {}


def _get_nc() -> bass.Bass:
    if "nc" not in _CACHE:
        _CACHE["nc"] = build_nc()
    return _CACHE["nc"]


def _pack_qT(query: np.ndarray) -> np.ndarray:
    return np.ascontiguousarray(
        query.T.reshape(KB, 128, BLOC * NCORES)  # [kb, p, B]
    )


def _prep(query, processed_memory, mask, Wq, v):
    """Build per-core input maps + scatter metadata for one gather pass set."""
    query = np.asarray(query, dtype=np.float32)
    pm = np.asarray(processed_memory)
    mask_b = np.asarray(mask).astype(bool)
    Wq = np.asarray(Wq, dtype=np.float32)
    v = np.asarray(v, dtype=np.float32)

    WqT16 = np.ascontiguousarray(Wq.T.astype(np.float16))      # (QD, AD)
    v16 = v.astype(np.float16)
    # oh[p, db, b, m] = v[db*128+p] iff m == b
    oh = np.zeros((128, DB, BLOC, BLOC), dtype=np.float16)
    for b_ in range(BLOC):
        oh[:, :, b_, b_] = v16.reshape(DB, 128).T
    padcol = (-10.0 * np.sign(v)).astype(np.float16)           # (AD,)

    idxs = [np.flatnonzero(~mask_b[gb]) for gb in range(B)]
    counts = np.array([len(ix) for ix in idxs])
    npass = max(1, int(np.ceil(counts.max() / TC)))

    qfull = query.T.reshape(KB, 128, B).transpose(1, 0, 2)     # (128, KB, B)

    pass_maps = []
    for p_ in range(npass):
        in_maps = []
        for i in range(NCORES):
            arr = np.empty((BLOC, DB, 128, TC), dtype=np.float16)
            arr[:] = padcol.reshape(1, DB, 128, 1)
            for b_ in range(BLOC):
                gb = i * BLOC + b_
                ix = idxs[gb][p_ * TC : (p_ + 1) * TC]
                if len(ix):
                    g = pm[gb, ix, :].astype(np.float16)       # (cnt, AD)
                    arr[b_, :, :, : len(ix)] = g.T.reshape(DB, 128, len(ix))
            sl = slice(i * BLOC, (i + 1) * BLOC)
            in_maps.append(
                {
                    "pm_g": arr,
                    "qT16": np.ascontiguousarray(
                        qfull[:, :, sl].reshape(128, KB * BLOC).astype(np.float16)
                    ),
                    "WqT16": WqT16,
                    "oh": oh,
                }
            )
        pass_maps.append(in_maps)
    return pass_maps, idxs, counts, npass


def run_spmd(in_maps, **kwargs):
    return run_bass_kernel_spmd(_get_nc(), in_maps, list(range(NCORES)), **kwargs)


def run_full(inputs: dict, **kwargs):
    """Run the full pipeline; returns (full_output, last_spmd_result)."""
    pass_maps, idxs, counts, npass = _prep(**inputs)
    results = []
    res = None
    for p_ in range(npass):
        res = run_spmd(pass_maps[p_], **kwargs)
        kwargs.pop("trace", None)  # only trace the first pass
        outs = np.concatenate(
            [res.results[i]["out"] for i in range(NCORES)], axis=0
        )  # (B, TC)
        sums = np.concatenate(
            [res.results[i]["rsum"] for i in range(NCORES)], axis=0
        )[:, 0]  # (B,)
        results.append((outs, sums))

    full = np.zeros((B, T), dtype=np.float32)
    for gb in range(B):
        cnt = counts[gb]
        if cnt == 0:
            full[gb, :] = 1.0 / T  # all masked -> uniform softmax
            continue
        if npass == 1:
            full[gb, idxs[gb]] = results[0][0][gb, :cnt]
        else:
            stot = sum(s[gb] for _, s in results if True)
            for p_ in range(npass):
                lo = p_ * TC
                ix = idxs[gb][lo : lo + TC]
                if len(ix):
                    o, s = results[p_]
                    full[gb, ix] = o[gb, : len(ix)] * (s[gb] / stot)
    return full, res


def kernel(query, processed_memory, mask, Wq, v) -> np.ndarray:
    full, _ = run_full(
        dict(query=query, processed_memory=processed_memory, mask=mask, Wq=Wq, v=v)
    )
    return full


# revision 15
# speedup vs baseline: 1.1719x; 1.1719x over previous
"""Bahdanau attention kernel for Trainium2 (8 NeuronCores, data-parallel over batch).

Reference computation (per batch row b):
    pq      = query @ Wq.T                       # (B, AD)
    hidden  = tanh(pq[:, None, :] + processed_memory)   # (B, T, AD)
    e       = einsum('btd,d->bt', hidden, v)     # (B, T)
    e       = where(mask, -1e30, e)
    out     = softmax(e, axis=1)

Key observation: ~50% of positions are masked (mask True -> softmax weight
exactly 0), so the host gathers only the unmasked columns of
processed_memory per batch (max count 2126 of 4096 for the reference data)
into a compact [AD, Tc=2176] slab, and scatters the compact softmax back
into the full [B, T] output (zeros at masked positions).  This halves HBM
traffic, tanh work and matmul work on the device.

Device strategy (per core, 8 batches):
  * gathered pm is host-transposed to [b, 2, 128, Tc] fp16 (AD=256 split in
    two partition blocks).  The per-d "+pq" add folds into the ScalarE tanh
    as a per-partition activation bias, v-weighted reduction over d runs on
    TensorE.
  * Energies accumulate directly into a PSUM tile [8, 2048] (+[8, 128]
    tail) with batch = partition: each matmul uses a one-hot stationary
    [128, 8] whose column b holds v (other columns zero), so it adds
    v.h on row b and +0 on the other rows.  No PSUM->SBUF strip copies, no
    SBUF relayout DMAs.
  * Pad columns (count_b..Tc) hold -10*sign(v_d): tanh saturates to
    -sign(v_d), so each pad contributes exp(-sum|v|) ~ 3e-6 to the softmax
    sum (rel err ~1e-7); pad outputs are discarded by the host scatter.
  * Softmax tail with batch on partitions: exp reads PSUM directly,
    row-sum + reciprocal + rescale on DVE, direct DMA to out[b, t].
"""

import sys

if "/opt/trn_rl_repo" not in sys.path:
    sys.path.insert(0, "/opt/trn_rl_repo")

import numpy as np

import concourse.bacc as bacc
import concourse.bass as bass
import concourse.tile as tile
from concourse import mybir
from concourse.bass_utils import run_bass_kernel_spmd

B, T, QD, AD = 64, 4096, 1024, 256
NCORES = 8
BLOC = B // NCORES  # batches per core
KB = QD // 128      # k-blocks for the pq matmul
DB = AD // 128      # d-blocks (partition blocks of AD)
F32 = mybir.dt.float32
F16 = mybir.dt.float16

MAIN = 2048         # main energies region (4 PSUM banks of 512 fp32)
TAIL = 80           # tail region (counts range ~1984..2126 <= 2128)
TC = MAIN + TAIL    # compact (gathered) time extent per batch
NCH = MAIN // 512   # 512-col chunks in the main region


def build_nc() -> bass.Bass:
    # Bacc (not plain Bass): its nop/event-semaphore lowering passes are what
    # let Tile-scheduled instructions carry multiple semaphore waits.
    nc = bacc.Bacc(None, target_bir_lowering=False)

    pm_g = nc.declare_dram_parameter("pm_g", [BLOC, DB, 128, TC], F16, isOutput=False)
    # wqq[p, kb*264 + j]: per k-block, 256 cols of Wq.T then 8 cols of
    # query.T (host-packed, partition-major) -> one DMA feeds all pq matmuls
    wqq = nc.declare_dram_parameter("wqq", [128, KB * (AD + BLOC)], F16, isOutput=False)
    # one-hot stationaries: oh[p, db, b, m] = v[db*128+p] if m == b else 0
    ohd = nc.declare_dram_parameter("oh", [128, DB, BLOC, BLOC], F16, isOutput=False)
    out = nc.declare_dram_parameter("out", [BLOC, TC], F16, isOutput=True)
    rsum = nc.declare_dram_parameter("rsum", [BLOC, 1], F32, isOutput=True)

    Tanh = mybir.ActivationFunctionType.Tanh
    Exp = mybir.ActivationFunctionType.Exp

    # energy matmul chunks: 512-col chunks (one PSUM bank each) + 128 tail
    chunks = [(c * 512, (c + 1) * 512) for c in range(NCH)] + [(MAIN, TC)]

    with tile.TileContext(nc) as tc:
        with (
            tc.tile_pool(name="singles", bufs=1) as singles,
            tc.tile_pool(name="pm", bufs=8) as pm_pool,
            tc.tile_pool(name="hid", bufs=6) as hid_pool,
            tc.tile_pool(name="epsum", bufs=1, space="PSUM") as epsum_pool,
            tc.tile_pool(name="ppsum", bufs=2, space="PSUM") as ppsum_pool,
        ):
            # ---- constant loads, ordered to minimize time-to-first-tanh:
            # wq d-block 0 + qt gate the d0 pq matmuls; the first pm tile is
            # split in half so tanh b0/d0 starts as early as possible ----
            wqq_sb = singles.tile([128, KB, AD + BLOC], F16)
            nc.sync.dma_start(
                out=wqq_sb,
                in_=wqq[:, :].rearrange("p (kb c) -> p kb c", c=AD + BLOC),
            )
            pm_sbs = {}
            HSPL = 1024
            pm_sbs[(0, 0)] = pm_pool.tile([128, TC], F16, name="pm_sb", tag="pm_sb")
            nc.sync.dma_start(out=pm_sbs[(0, 0)][:, 0:HSPL], in_=pm_g[0, 0][:, 0:HSPL])
            nc.sync.dma_start(out=pm_sbs[(0, 0)][:, HSPL:TC], in_=pm_g[0, 0][:, HSPL:TC])
            pm_sbs[(0, 1)] = pm_pool.tile([128, TC], F16, name="pm_sb", tag="pm_sb")
            nc.sync.dma_start(out=pm_sbs[(0, 1)], in_=pm_g[0, 1])
            oh_sb = singles.tile([128, DB, BLOC, BLOC], F16)
            nc.sync.dma_start(out=oh_sb, in_=ohd[:, :, :, :])
            pm_m = {}
            for b in range(1, BLOC):
                if b < BLOC - 1:
                    # merged per-batch tile: DVE adds the pq bias in place,
                    # then one bias-free tanh covers both d-blocks
                    pm_m[b] = pm_pool.tile([128, DB, TC], F16, name="pm_m", tag="pm_m")
                    nc.sync.dma_start(
                        out=pm_m[b], in_=pm_g[b].rearrange("db p t -> p db t")
                    )
                else:
                    for d in range(DB):
                        pm_sbs[(b, d)] = pm_pool.tile([128, TC], F16, name="pm_sb", tag="pm_sb")
                        nc.sync.dma_start(out=pm_sbs[(b, d)], in_=pm_g[b, d])

            # ---- energies PSUM accumulator (batch = partition row) ----
            # [8, 2128] fp32 spans 4.2 banks; every matmul chunk below stays
            # inside a single 512-fp32 bank region.
            ep = epsum_pool.tile([BLOC, TC], F32, tag="ep")
            pq_sb = singles.tile([128, DB, BLOC], F32)

            def emit_pq(d):
                # pq = Wq @ query.T for one d-block, laid out [d % 128, b]
                ppq = ppsum_pool.tile([128, BLOC], F32, tag="ppq", name="ppq")
                for k in range(KB):
                    nc.tensor.matmul(
                        ppq,
                        lhsT=wqq_sb[:, k, d * 128 : (d + 1) * 128],
                        rhs=wqq_sb[:, k, AD : AD + BLOC],
                        start=(k == 0),
                        stop=(k == KB - 1),
                    )
                # DVE copy: keeps the pq bias copies off the tanh (ACT) stream
                nc.vector.tensor_copy(out=pq_sb[:, d, :], in_=ppq)

            def emit_bd(b, d, split):
                # biased tanh for one (batch, d-block) + its energy matmuls
                h = hid_pool.tile([128, TC], F16, name="h", tag="h")
                first = b == 0 and d == 0
                last = b == BLOC - 1 and d == DB - 1
                parts = ((0, HSPL), (HSPL, TC)) if split else ((0, TC),)
                for lo2, hi2 in parts:
                    nc.scalar.activation(
                        out=h[:, lo2:hi2],
                        in_=pm_sbs[(b, d)][:, lo2:hi2],
                        func=Tanh,
                        bias=pq_sb[:, d, b : b + 1],
                        scale=1.0,
                    )
                for lo, hi in chunks:
                    nc.tensor.matmul(
                        ep[:, lo:hi],
                        lhsT=oh_sb[:, d, b, :],
                        rhs=h[:, lo:hi],
                        start=first,
                        stop=last,
                    )

            # batch 0: pq d1 is computed between the two d-blocks so the
            # d1 bias copy never delays the first tanh (ACT is in-order)
            emit_pq(0)
            emit_bd(0, 0, split=True)
            emit_pq(1)
            emit_bd(0, 1, split=False)

            # batches 1..6: DVE adds the pq bias in place, then one merged
            # bias-free tanh covers both d-blocks (saves ACT instr overhead)
            for b in range(1, BLOC - 1):
                for d in range(DB):
                    nc.vector.tensor_scalar_add(
                        out=pm_m[b][:, d, :],
                        in0=pm_m[b][:, d, :],
                        scalar1=pq_sb[:, d, b : b + 1],
                    )
                hm = hid_pool.tile([128, DB, TC], F16, name="hm", tag="hm")
                nc.scalar.activation(out=hm, in_=pm_m[b], func=Tanh)
                for d in range(DB):
                    for lo, hi in chunks:
                        nc.tensor.matmul(
                            ep[:, lo:hi],
                            lhsT=oh_sb[:, d, b, :],
                            rhs=hm[:, d, lo:hi],
                            start=False,
                            stop=False,
                        )

            # batch 7: split the last tanh so its chunk-0/1 matmuls overlap
            # the second tanh half (shortens the matmul tail before exp)
            emit_bd(BLOC - 1, 0, split=False)
            emit_bd(BLOC - 1, 1, split=True)

            # ---- softmax tail: batch already on partitions; exp reads PSUM
            # directly and its row-sum is fused via accum_out ----
            # exp(e) <= exp(~3) ~ 20 for this data (|e| worst-case bound 12.8
            # needs 17 sigma) so fp16 work is safe and gets DVE 2x rescale
            work = singles.tile([BLOC, TC], F16)
            outw = singles.tile([BLOC, TC], F16)
            cs = singles.tile([BLOC, 1], F32)
            nc.scalar.activation(out=work, in_=ep, func=Exp, accum_out=cs)
            nc.sync.dma_start(out=rsum[:, :], in_=cs)
            rinv = singles.tile([BLOC, 1], F32)
            nc.vector.reciprocal(out=rinv, in_=cs)
            nc.vector.tensor_scalar_mul(out=outw, in0=work, scalar1=rinv)
            nc.sync.dma_start(out=out[:, :], in_=outw)

    # Run the Bacc lowering passes (move_matmul_waits_to_ldweights,
    # generate_event_semaphores, alloc_regs, ...) — run_bass_via_pjrt takes
    # the module as-is and walrus rejects unlowered multi-wait instructions.
    nc.finalize()
    return nc


_CACHE: dict = {}


def _get_nc() -> bass.Bass:
    if "nc" not in _CACHE:
        _CACHE["nc"] = build_nc()
    return _CACHE["nc"]


def _pack_qT(query: np.ndarray) -> np.ndarray:
    return np.ascontiguousarray(
        query.T.reshape(KB, 128, BLOC * NCORES)  # [kb, p, B]
    )


def _prep(query, processed_memory, mask, Wq, v):
    """Build per-core input maps + scatter metadata for one gather pass set."""
    query = np.asarray(query, dtype=np.float32)
    pm = np.asarray(processed_memory)
    mask_b = np.asarray(mask).astype(bool)
    Wq = np.asarray(Wq, dtype=np.float32)
    v = np.asarray(v, dtype=np.float32)

    WqT16 = Wq.T.astype(np.float16)                            # (QD, AD)
    v16 = v.astype(np.float16)
    wq_blocks = WqT16.reshape(KB, 128, AD)                     # [kb, p, d]
    # oh[p, db, b, m] = v[db*128+p] iff m == b
    oh = np.zeros((128, DB, BLOC, BLOC), dtype=np.float16)
    for b_ in range(BLOC):
        oh[:, :, b_, b_] = v16.reshape(DB, 128).T
    padcol = (-10.0 * np.sign(v)).astype(np.float16)           # (AD,)

    idxs = [np.flatnonzero(~mask_b[gb]) for gb in range(B)]
    counts = np.array([len(ix) for ix in idxs])
    npass = max(1, int(np.ceil(counts.max() / TC)))

    qfull = query.T.reshape(KB, 128, B).transpose(1, 0, 2)     # (128, KB, B)

    pass_maps = []
    for p_ in range(npass):
        in_maps = []
        for i in range(NCORES):
            arr = np.empty((BLOC, DB, 128, TC), dtype=np.float16)
            arr[:] = padcol.reshape(1, DB, 128, 1)
            for b_ in range(BLOC):
                gb = i * BLOC + b_
                ix = idxs[gb][p_ * TC : (p_ + 1) * TC]
                if len(ix):
                    g = pm[gb, ix, :].astype(np.float16)       # (cnt, AD)
                    arr[b_, :, :, : len(ix)] = g.T.reshape(DB, 128, len(ix))
            sl = slice(i * BLOC, (i + 1) * BLOC)
            qb = qfull[:, :, sl].astype(np.float16)            # [p, kb, b]
            wqq_h = np.concatenate(
                [wq_blocks.transpose(1, 0, 2), qb], axis=2
            )                                                   # [p, kb, 264]
            in_maps.append(
                {
                    "pm_g": arr,
                    "wqq": np.ascontiguousarray(
                        wqq_h.reshape(128, KB * (AD + BLOC))
                    ),
                    "oh": oh,
                }
            )
        pass_maps.append(in_maps)
    return pass_maps, idxs, counts, npass


def run_spmd(in_maps, **kwargs):
    return run_bass_kernel_spmd(_get_nc(), in_maps, list(range(NCORES)), **kwargs)


def run_full(inputs: dict, **kwargs):
    """Run the full pipeline; returns (full_output, last_spmd_result)."""
    pass_maps, idxs, counts, npass = _prep(**inputs)
    results = []
    res = None
    for p_ in range(npass):
        res = run_spmd(pass_maps[p_], **kwargs)
        kwargs.pop("trace", None)  # only trace the first pass
        outs = np.concatenate(
            [res.results[i]["out"] for i in range(NCORES)], axis=0
        )  # (B, TC)
        sums = np.concatenate(
            [res.results[i]["rsum"] for i in range(NCORES)], axis=0
        )[:, 0]  # (B,)
        results.append((outs, sums))

    full = np.zeros((B, T), dtype=np.float32)
    for gb in range(B):
        cnt = counts[gb]
        if cnt == 0:
            full[gb, :] = 1.0 / T  # all masked -> uniform softmax
            continue
        if npass == 1:
            full[gb, idxs[gb]] = results[0][0][gb, :cnt]
        else:
            stot = sum(s[gb] for _, s in results if True)
            for p_ in range(npass):
                lo = p_ * TC
                ix = idxs[gb][lo : lo + TC]
                if len(ix):
                    o, s = results[p_]
                    full[gb, ix] = o[gb, : len(ix)] * (s[gb] / stot)
    return full, res


def kernel(query, processed_memory, mask, Wq, v) -> np.ndarray:
    full, _ = run_full(
        dict(query=query, processed_memory=processed_memory, mask=mask, Wq=Wq, v=v)
    )
    return full


# revision 16
# speedup vs baseline: 1.2164x; 1.0379x over previous
"""Bahdanau attention kernel for Trainium2 (8 NeuronCores, data-parallel over batch).

Reference computation (per batch row b):
    pq      = query @ Wq.T                       # (B, AD)
    hidden  = tanh(pq[:, None, :] + processed_memory)   # (B, T, AD)
    e       = einsum('btd,d->bt', hidden, v)     # (B, T)
    e       = where(mask, -1e30, e)
    out     = softmax(e, axis=1)

Key observation: ~50% of positions are masked (mask True -> softmax weight
exactly 0), so the host gathers only the unmasked columns of
processed_memory per batch (max count 2126 of 4096 for the reference data)
into a compact [AD, Tc=2176] slab, and scatters the compact softmax back
into the full [B, T] output (zeros at masked positions).  This halves HBM
traffic, tanh work and matmul work on the device.

Device strategy (per core, 8 batches):
  * gathered pm is host-transposed to [b, 2, 128, Tc] fp16 (AD=256 split in
    two partition blocks).  The per-d "+pq" add folds into the ScalarE tanh
    as a per-partition activation bias, v-weighted reduction over d runs on
    TensorE.
  * Energies accumulate directly into a PSUM tile [8, 2048] (+[8, 128]
    tail) with batch = partition: each matmul uses a one-hot stationary
    [128, 8] whose column b holds v (other columns zero), so it adds
    v.h on row b and +0 on the other rows.  No PSUM->SBUF strip copies, no
    SBUF relayout DMAs.
  * Pad columns (count_b..Tc) hold -10*sign(v_d): tanh saturates to
    -sign(v_d), so each pad contributes exp(-sum|v|) ~ 3e-6 to the softmax
    sum (rel err ~1e-7); pad outputs are discarded by the host scatter.
  * Softmax tail with batch on partitions: exp reads PSUM directly,
    row-sum + reciprocal + rescale on DVE, direct DMA to out[b, t].
"""

import sys

if "/opt/trn_rl_repo" not in sys.path:
    sys.path.insert(0, "/opt/trn_rl_repo")

import numpy as np

import concourse.bacc as bacc
import concourse.bass as bass
import concourse.tile as tile
from concourse import mybir
from concourse.bass_utils import run_bass_kernel_spmd

B, T, QD, AD = 64, 4096, 1024, 256
NCORES = 8
BLOC = B // NCORES  # batches per core
KB = QD // 128      # k-blocks for the pq matmul
DB = AD // 128      # d-blocks (partition blocks of AD)
F32 = mybir.dt.float32
F16 = mybir.dt.float16

MAIN = 2048         # main energies region (4 PSUM banks of 512 fp32)
TAIL = 80           # tail region (counts range ~1984..2126 <= 2128)
TC = MAIN + TAIL    # compact (gathered) time extent per batch
NCH = MAIN // 512   # 512-col chunks in the main region


def build_nc() -> bass.Bass:
    # Bacc (not plain Bass): its nop/event-semaphore lowering passes are what
    # let Tile-scheduled instructions carry multiple semaphore waits.
    nc = bacc.Bacc(None, target_bir_lowering=False)

    pm_g = nc.declare_dram_parameter("pm_g", [BLOC, DB, 128, TC], F16, isOutput=False)
    # wqq[p, kb*264 + j]: per k-block, 256 cols of Wq.T then 8 cols of
    # query.T (host-packed, partition-major) -> one DMA feeds all pq matmuls
    wqq = nc.declare_dram_parameter("wqq", [128, KB * (AD + BLOC)], F16, isOutput=False)
    # one-hot stationaries: oh[p, db, b, m] = v[db*128+p] if m == b else 0
    ohd = nc.declare_dram_parameter("oh", [128, DB, BLOC, BLOC], F16, isOutput=False)
    out = nc.declare_dram_parameter("out", [BLOC, TC], F16, isOutput=True)
    rsum = nc.declare_dram_parameter("rsum", [BLOC, 1], F32, isOutput=True)

    Tanh = mybir.ActivationFunctionType.Tanh
    Exp = mybir.ActivationFunctionType.Exp

    # energy matmul chunks: 512-col chunks (one PSUM bank each) + 128 tail
    chunks = [(c * 512, (c + 1) * 512) for c in range(NCH)] + [(MAIN, TC)]

    with tile.TileContext(nc) as tc:
        with (
            tc.tile_pool(name="singles", bufs=1) as singles,
            tc.tile_pool(name="pm", bufs=8) as pm_pool,
            tc.tile_pool(name="hid", bufs=6) as hid_pool,
            tc.tile_pool(name="epsum", bufs=1, space="PSUM") as epsum_pool,
            tc.tile_pool(name="ppsum", bufs=2, space="PSUM") as ppsum_pool,
        ):
            # ---- constant loads, ordered to minimize time-to-first-tanh:
            # wq d-block 0 + qt gate the d0 pq matmuls; the first pm tile is
            # split in half so tanh b0/d0 starts as early as possible ----
            wqq_sb = singles.tile([128, KB, AD + BLOC], F16)
            wqq_r = wqq[:, :].rearrange("p (kb c) -> p kb c", c=AD + BLOC)
            nc.sync.dma_start(out=wqq_sb[:, 0 : KB // 2, :], in_=wqq_r[:, 0 : KB // 2, :])
            nc.sync.dma_start(out=wqq_sb[:, KB // 2 : KB, :], in_=wqq_r[:, KB // 2 : KB, :])
            pm_sbs = {}
            HSPL = 1024
            pm_sbs[(0, 0)] = pm_pool.tile([128, TC], F16, name="pm_sb", tag="pm_sb")
            nc.sync.dma_start(out=pm_sbs[(0, 0)][:, 0:HSPL], in_=pm_g[0, 0][:, 0:HSPL])
            nc.sync.dma_start(out=pm_sbs[(0, 0)][:, HSPL:TC], in_=pm_g[0, 0][:, HSPL:TC])
            pm_sbs[(0, 1)] = pm_pool.tile([128, TC], F16, name="pm_sb", tag="pm_sb")
            nc.sync.dma_start(out=pm_sbs[(0, 1)], in_=pm_g[0, 1])
            oh_sb = singles.tile([128, DB, BLOC, BLOC], F16)
            nc.sync.dma_start(out=oh_sb, in_=ohd[:, :, :, :])
            pm_m = {}
            MERGED = set(range(3, BLOC - 1))
            for b in range(1, BLOC):
                if b in MERGED:
                    # merged per-batch tile: DVE adds the pq bias in place,
                    # then one bias-free tanh covers both d-blocks
                    pm_m[b] = pm_pool.tile([128, DB, TC], F16, name="pm_m", tag="pm_m")
                    nc.sync.dma_start(
                        out=pm_m[b], in_=pm_g[b].rearrange("db p t -> p db t")
                    )
                else:
                    for d in range(DB):
                        pm_sbs[(b, d)] = pm_pool.tile([128, TC], F16, name="pm_sb", tag="pm_sb")
                        nc.sync.dma_start(out=pm_sbs[(b, d)], in_=pm_g[b, d])

            # ---- energies PSUM accumulator (batch = partition row) ----
            # [8, 2128] fp32 spans 4.2 banks; every matmul chunk below stays
            # inside a single 512-fp32 bank region.
            ep = epsum_pool.tile([BLOC, TC], F32, tag="ep")
            pq_sb = singles.tile([128, DB, BLOC], F32)

            def emit_pq(d):
                # pq = Wq @ query.T for one d-block, laid out [d % 128, b]
                ppq = ppsum_pool.tile([128, BLOC], F32, tag="ppq", name="ppq")
                for k in range(KB):
                    nc.tensor.matmul(
                        ppq,
                        lhsT=wqq_sb[:, k, d * 128 : (d + 1) * 128],
                        rhs=wqq_sb[:, k, AD : AD + BLOC],
                        start=(k == 0),
                        stop=(k == KB - 1),
                    )
                # DVE copy: keeps the pq bias copies off the tanh (ACT) stream
                nc.vector.tensor_copy(out=pq_sb[:, d, :], in_=ppq)

            def emit_bd(b, d, split):
                # biased tanh for one (batch, d-block) + its energy matmuls
                h = hid_pool.tile([128, TC], F16, name="h", tag="h")
                first = b == 0 and d == 0
                last = b == BLOC - 1 and d == DB - 1
                parts = ((0, HSPL), (HSPL, TC)) if split else ((0, TC),)
                for lo2, hi2 in parts:
                    nc.scalar.activation(
                        out=h[:, lo2:hi2],
                        in_=pm_sbs[(b, d)][:, lo2:hi2],
                        func=Tanh,
                        bias=pq_sb[:, d, b : b + 1],
                        scale=1.0,
                    )
                for lo, hi in chunks:
                    nc.tensor.matmul(
                        ep[:, lo:hi],
                        lhsT=oh_sb[:, d, b, :],
                        rhs=h[:, lo:hi],
                        start=first,
                        stop=last,
                    )

            # batch 0: pq d1 is computed between the two d-blocks so the
            # d1 bias copy never delays the first tanh (ACT is in-order)
            emit_pq(0)
            emit_bd(0, 0, split=True)
            emit_pq(1)
            emit_bd(0, 1, split=False)

            # early batches stay on the per-d-block path (their smaller DMAs
            # land sooner, keeping the tanh stream gapless while the pipeline
            # fills); later batches use the DVE-bias + merged-tanh path
            for b in range(1, BLOC - 1):
                if b not in MERGED:
                    emit_bd(b, 0, split=False)
                    emit_bd(b, 1, split=False)
                    continue
                for d in range(DB):
                    nc.vector.tensor_scalar_add(
                        out=pm_m[b][:, d, :],
                        in0=pm_m[b][:, d, :],
                        scalar1=pq_sb[:, d, b : b + 1],
                    )
                hm = hid_pool.tile([128, DB, TC], F16, name="hm", tag="hm")
                nc.scalar.activation(out=hm, in_=pm_m[b], func=Tanh)
                for d in range(DB):
                    for lo, hi in chunks:
                        nc.tensor.matmul(
                            ep[:, lo:hi],
                            lhsT=oh_sb[:, d, b, :],
                            rhs=hm[:, d, lo:hi],
                            start=False,
                            stop=False,
                        )

            # batch 7: split the last tanh so its chunk-0/1 matmuls overlap
            # the second tanh half (shortens the matmul tail before exp)
            emit_bd(BLOC - 1, 0, split=False)
            emit_bd(BLOC - 1, 1, split=True)

            # ---- softmax tail: batch already on partitions; exp reads PSUM
            # directly and its row-sum is fused via accum_out ----
            # exp(e) <= exp(~3) ~ 20 for this data (|e| worst-case bound 12.8
            # needs 17 sigma) so fp16 work is safe and gets DVE 2x rescale
            work = singles.tile([BLOC, TC], F16)
            outw = singles.tile([BLOC, TC], F16)
            cs = singles.tile([BLOC, 1], F32)
            nc.scalar.activation(out=work, in_=ep, func=Exp, accum_out=cs)
            nc.sync.dma_start(out=rsum[:, :], in_=cs)
            rinv = singles.tile([BLOC, 1], F32)
            nc.vector.reciprocal(out=rinv, in_=cs)
            nc.vector.tensor_scalar_mul(out=outw, in0=work, scalar1=rinv)
            nc.sync.dma_start(out=out[:, :], in_=outw)

    # Run the Bacc lowering passes (move_matmul_waits_to_ldweights,
    # generate_event_semaphores, alloc_regs, ...) — run_bass_via_pjrt takes
    # the module as-is and walrus rejects unlowered multi-wait instructions.
    nc.finalize()
    return nc


_CACHE: dict = {}


def _get_nc() -> bass.Bass:
    if "nc" not in _CACHE:
        _CACHE["nc"] = build_nc()
    return _CACHE["nc"]


def _pack_qT(query: np.ndarray) -> np.ndarray:
    return np.ascontiguousarray(
        query.T.reshape(KB, 128, BLOC * NCORES)  # [kb, p, B]
    )


def _prep(query, processed_memory, mask, Wq, v):
    """Build per-core input maps + scatter metadata for one gather pass set."""
    query = np.asarray(query, dtype=np.float32)
    pm = np.asarray(processed_memory)
    mask_b = np.asarray(mask).astype(bool)
    Wq = np.asarray(Wq, dtype=np.float32)
    v = np.asarray(v, dtype=np.float32)

    WqT16 = Wq.T.astype(np.float16)                            # (QD, AD)
    v16 = v.astype(np.float16)
    wq_blocks = WqT16.reshape(KB, 128, AD)                     # [kb, p, d]
    # oh[p, db, b, m] = v[db*128+p] iff m == b
    oh = np.zeros((128, DB, BLOC, BLOC), dtype=np.float16)
    for b_ in range(BLOC):
        oh[:, :, b_, b_] = v16.reshape(DB, 128).T
    padcol = (-10.0 * np.sign(v)).astype(np.float16)           # (AD,)

    idxs = [np.flatnonzero(~mask_b[gb]) for gb in range(B)]
    counts = np.array([len(ix) for ix in idxs])
    npass = max(1, int(np.ceil(counts.max() / TC)))

    qfull = query.T.reshape(KB, 128, B).transpose(1, 0, 2)     # (128, KB, B)

    pass_maps = []
    for p_ in range(npass):
        in_maps = []
        for i in range(NCORES):
            arr = np.empty((BLOC, DB, 128, TC), dtype=np.float16)
            arr[:] = padcol.reshape(1, DB, 128, 1)
            for b_ in range(BLOC):
                gb = i * BLOC + b_
                ix = idxs[gb][p_ * TC : (p_ + 1) * TC]
                if len(ix):
                    g = pm[gb, ix, :].astype(np.float16)       # (cnt, AD)
                    arr[b_, :, :, : len(ix)] = g.T.reshape(DB, 128, len(ix))
            sl = slice(i * BLOC, (i + 1) * BLOC)
            qb = qfull[:, :, sl].astype(np.float16)            # [p, kb, b]
            wqq_h = np.concatenate(
                [wq_blocks.transpose(1, 0, 2), qb], axis=2
            )                                                   # [p, kb, 264]
            in_maps.append(
                {
                    "pm_g": arr,
                    "wqq": np.ascontiguousarray(
                        wqq_h.reshape(128, KB * (AD + BLOC))
                    ),
                    "oh": oh,
                }
            )
        pass_maps.append(in_maps)
    return pass_maps, idxs, counts, npass


def run_spmd(in_maps, **kwargs):
    return run_bass_kernel_spmd(_get_nc(), in_maps, list(range(NCORES)), **kwargs)


def run_full(inputs: dict, **kwargs):
    """Run the full pipeline; returns (full_output, last_spmd_result)."""
    pass_maps, idxs, counts, npass = _prep(**inputs)
    results = []
    res = None
    for p_ in range(npass):
        res = run_spmd(pass_maps[p_], **kwargs)
        kwargs.pop("trace", None)  # only trace the first pass
        outs = np.concatenate(
            [res.results[i]["out"] for i in range(NCORES)], axis=0
        )  # (B, TC)
        sums = np.concatenate(
            [res.results[i]["rsum"] for i in range(NCORES)], axis=0
        )[:, 0]  # (B,)
        results.append((outs, sums))

    full = np.zeros((B, T), dtype=np.float32)
    for gb in range(B):
        cnt = counts[gb]
        if cnt == 0:
            full[gb, :] = 1.0 / T  # all masked -> uniform softmax
            continue
        if npass == 1:
            full[gb, idxs[gb]] = results[0][0][gb, :cnt]
        else:
            stot = sum(s[gb] for _, s in results if True)
            for p_ in range(npass):
                lo = p_ * TC
                ix = idxs[gb][lo : lo + TC]
                if len(ix):
                    o, s = results[p_]
                    full[gb, ix] = o[gb, : len(ix)] * (s[gb] / stot)
    return full, res


def kernel(query, processed_memory, mask, Wq, v) -> np.ndarray:
    full, _ = run_full(
        dict(query=query, processed_memory=processed_memory, mask=mask, Wq=Wq, v=v)
    )
    return full
